# revision 1
# baseline (speedup 1.0000x reference)
"""Distributed KNN online evaluator kernel for 8 trn2 NeuronCores.

Device side (SPMD over 8 cores, bank sharded over N):
  - bf16 matmul sim tiles (queries stationary) -> f32 PSUM
  - blockmax-of-8 reduction (DVE tensor_tensor max tree / ACT copy assist)
  - DMA out per-(query, block) maxima as bf16

Host side:
  - adaptive drill-down: select blocks whose blockmax could contain a
    global top-K sim, recompute those sims exactly in f32, take top-K
  - verified: every unselected block provably below the top-K threshold
    (margin covers bf16/matmul fuzz); expands selection until proven
  - class votes with inf weights degenerate to membership -> output is
    [voted classes asc, unvoted classes asc] per query
"""

import numpy as np
import ml_dtypes

import concourse.bass as bass
import concourse.mybir as mybir
from concourse.bass_utils import run_bass_kernel_spmd

BF16 = ml_dtypes.bfloat16

N_CORES = 8
B = 256  # queries
D = 128  # feature dim
N_TOTAL = 200000
N_SHARD = N_TOTAL // N_CORES  # 25000
GROUP = 2048  # columns per psum group (4 banks of 512 f32)
N_GROUPS = 13  # per chunk: 13 * 2048 = 26624 >= 25000
NCOL = N_GROUPS * GROUP  # padded shard width
BLK = 8  # block size of the device blockmax
SLOTS_PER_GROUP = GROUP // BLK  # 256
SLOTS = N_GROUPS * SLOTS_PER_GROUP  # 3328 per chunk
K = 200
NUM_CLASSES = 1000
MARGIN = 1.5  # device blockmax fuzz bound vs exact f32 sim (bf16 inputs)

# act_mask[i]: step i evacuated by ACT (copy+DVE fold) vs DVE (TT from psum)
N_STEPS = 2 * N_GROUPS  # (chunk, group) pairs
ACT_MASK = [(i % 3) != 2 for i in range(N_STEPS)]

_NC_CACHE = None


def _build_nc():
    nc = bass.Bass("TRN2", target_bir_lowering=False, debug=False,
                   num_devices=N_CORES)
    qT = nc.dram_tensor("qT", [D, B], mybir.dt.bfloat16,
                        kind="ExternalInput").ap()
    bankT = nc.dram_tensor("bankT", [D, NCOL], mybir.dt.bfloat16,
                           kind="ExternalInput").ap()
    out = nc.dram_tensor("blockmax", [B, SLOTS], mybir.dt.bfloat16,
                         kind="ExternalOutput").ap()

    nA = np.cumsum(ACT_MASK)          # A-steps completed up to i (incl)
    nD = np.cumsum([not m for m in ACT_MASK])

    with (
        nc.sbuf_tensor([D, B], mybir.dt.bfloat16) as qs,
        nc.sbuf_tensor([D, 3 * GROUP], mybir.dt.bfloat16) as banks,  # ring 3
        nc.psum_tensor([128, 2 * GROUP], mybir.dt.float32) as psum,  # ring 2
        nc.sbuf_tensor([128, 2 * GROUP], mybir.dt.bfloat16) as stage,  # ring 2
        nc.sbuf_tensor([128, 2 * (GROUP // 2)], mybir.dt.bfloat16) as l1,
        nc.sbuf_tensor([128, 2 * (GROUP // 4)], mybir.dt.bfloat16) as l2,
        nc.sbuf_tensor([128, 2 * SLOTS], mybir.dt.bfloat16) as obuf,
        nc.semaphore() as dma_sem,
        nc.semaphore() as mm_sem,
        nc.semaphore() as evacA,   # ACT copies done
        nc.semaphore() as f1A,     # DVE folds of stage done (frees stage)
        nc.semaphore() as fold_sem,
        nc.Block() as block,
    ):
        def step_cg(i):
            return i % 2, i // 2  # chunk, bank-group

        @block.sync
        def _(sync):
            sync.dma_start(qs[:], qT).then_inc(dma_sem, 16)
            for bg in range(N_GROUPS):
                if bg >= 3:  # bank ring slot reuse: groups 2bg', 2bg'+1 MM'd
                    sync.wait_ge(mm_sem, 2 * (bg - 3) + 2)
                sync.dma_start(banks[:, (bg % 3) * GROUP:(bg % 3 + 1) * GROUP],
                               bankT[:, bg * GROUP:(bg + 1) * GROUP]
                               ).then_inc(dma_sem, 16)
            for i in range(N_STEPS):
                c, bg = step_cg(i)
                lo = bg * SLOTS_PER_GROUP
                hi = (bg + 1) * SLOTS_PER_GROUP
                sync.wait_ge(fold_sem, i + 1)
                sync.dma_start(out[c * 128:(c + 1) * 128, lo:hi],
                               obuf[:, c * SLOTS + lo:c * SLOTS + hi]
                               ).then_inc(dma_sem, 16)

        @block.tensor
        def _(tensor):
            for i in range(N_STEPS):
                c, bg = step_cg(i)
                tensor.wait_ge(dma_sem, 16 * (bg + 2))  # qT + banks 0..bg
                if i >= 2:  # psum ring slot i%2 last used at step i-2
                    j = i - 2
                    if ACT_MASK[j]:
                        tensor.wait_ge(evacA, nA[j])
                    else:
                        tensor.wait_ge(fold_sem, j + 1)
                s = (i % 2) * GROUP
                bb = (bg % 3) * GROUP
                for k in range(4):
                    mm = tensor.matmul(
                        psum[:, s + k * 512: s + (k + 1) * 512],
                        lhsT=qs[:, c * 128:(c + 1) * 128],
                        rhs=banks[:, bb + k * 512: bb + (k + 1) * 512],
                        start=True, stop=True)
                    if k == 3:
                        mm.then_inc(mm_sem, 1)

        @block.scalar
        def _(scalar):
            na = 0
            for i in range(N_STEPS):
                if not ACT_MASK[i]:
                    continue
                c, bg = step_cg(i)
                if na >= 2:  # stage ring slot reuse: wait DVE f1 of prev use
                    scalar.wait_ge(f1A, na - 1)
                scalar.wait_ge(mm_sem, i + 1)
                s = (i % 2) * GROUP
                ss = (na % 2) * GROUP
                scalar.copy(stage[:, ss:ss + GROUP],
                            psum[:, s:s + GROUP]).then_inc(evacA, 1)
                na += 1

        @block.vector
        def _(vector):
            MAX = mybir.AluOpType.max
            na = 0
            for i in range(N_STEPS):
                c, bg = step_cg(i)
                r = (i % 2)
                h1 = GROUP // 2   # 1024
                h2 = GROUP // 4   # 512
                l1s = l1[:, r * h1:(r + 1) * h1]
                l2s = l2[:, r * h2:(r + 1) * h2]
                oslot = obuf[:, c * SLOTS + bg * SLOTS_PER_GROUP:
                             c * SLOTS + (bg + 1) * SLOTS_PER_GROUP]
                if ACT_MASK[i]:
                    vector.wait_ge(evacA, na + 1)
                    ss = (na % 2) * GROUP
                    vector.tensor_tensor(
                        out=l1s, in0=stage[:, ss:ss + h1],
                        in1=stage[:, ss + h1:ss + GROUP],
                        op=MAX).then_inc(f1A, 1)
                    na += 1
                    vector.tensor_tensor(out=l2s, in0=l1s[:, :h2],
                                         in1=l1s[:, h2:], op=MAX)
                    vector.tensor_tensor(out=oslot, in0=l2s[:, :h2 // 2],
                                         in1=l2s[:, h2 // 2:],
                                         op=MAX).then_inc(fold_sem, 1)
                else:
                    vector.wait_ge(mm_sem, i + 1)
                    s = r * GROUP
                    vector.tensor_reduce(
                        out=oslot,
                        in_=psum[:, s:s + GROUP].rearrange(
                            "p (b w) -> p b w", w=BLK),
                        axis=mybir.AxisListType.X,
                        op=MAX,
                    ).then_inc(fold_sem, 1)
    return nc


def _get_nc():
    global _NC_CACHE
    if _NC_CACHE is None:
        _NC_CACHE = _build_nc()
    return _NC_CACHE


def _run_device(query_feature, feature_bank, trace=False):
    qT = np.ascontiguousarray(query_feature.astype(np.float32).T
                              ).astype(BF16)  # [128, 256]
    in_maps = []
    for i in range(N_CORES):
        shard = feature_bank[i * N_SHARD:(i + 1) * N_SHARD].astype(np.float32)
        bt = np.zeros((D, NCOL), dtype=BF16)
        bt[:, :N_SHARD] = np.ascontiguousarray(shard.T).astype(BF16)
        in_maps.append({"qT": qT, "bankT": bt})
    nc = _get_nc()
    res = run_bass_kernel_spmd(nc, in_maps, list(range(N_CORES)), trace=trace)
    bm = np.stack([res.results[i]["blockmax"].astype(np.float32)
                   for i in range(N_CORES)])  # [8, 256, SLOTS]
    return bm, res


def _slot_rows(c):
    """Row preimage of each slot for chunk c: [SLOTS, BLK] local col idx.

    ACT groups (fold tree): slot (bg, j) covers bg*2048 + j + 256*k, k<8.
    DVE groups (pool-8):    slot (bg, j) covers bg*2048 + 8*j + k, k<8.
    """
    rows = np.empty((SLOTS, BLK), dtype=np.int64)
    j = np.arange(SLOTS_PER_GROUP)
    k = np.arange(BLK)
    for bg in range(N_GROUPS):
        if ACT_MASK[2 * bg + c]:
            blk = j[:, None] + 256 * k[None, :]
        else:
            blk = 8 * j[:, None] + k[None, :]
        rows[bg * SLOTS_PER_GROUP + j] = bg * GROUP + blk
    return rows  # local column indices within a core's padded shard


def _host_topk(bm, query_feature, feature_bank, nsel=96):
    """bm: [8, 256, SLOTS] f32 device blockmaxima. Returns top-K indices
    [B, K] into the full bank, matching f32 jax top_k semantics.

    Vectorized drill-down: per round, gather the top-nb blocks per query,
    recompute their sims exactly in f32, and accept a query once every
    unselected block is provably (within MARGIN) below its K-th value.
    """
    q = query_feature.astype(np.float32)
    fb = feature_bank.astype(np.float32)
    grow_flat = np.empty((2, N_CORES * SLOTS, BLK), dtype=np.int64)
    for ch in range(2):
        srows = _slot_rows(ch)  # [SLOTS, BLK] local cols
        for cidx in range(N_CORES):
            g = srows + cidx * N_SHARD
            g[srows >= N_SHARD] = N_TOTAL  # padding -> sentinel row
            grow_flat[ch, cidx * SLOTS:(cidx + 1) * SLOTS] = g
    bm_flat = bm.transpose(1, 0, 2).reshape(B, N_CORES * SLOTS)
    fb_pad = np.vstack([fb, np.zeros((1, D), np.float32)])

    order = np.argsort(-bm_flat, axis=1)
    sel_sorted = np.take_along_axis(bm_flat, order, axis=1)
    topk_idx = np.empty((B, K), dtype=np.int64)
    pending = np.arange(B)
    nb = nsel
    while len(pending):
        nb = min(nb, bm_flat.shape[1])
        rows = grow_flat[(pending // 128)[:, None],
                         order[pending, :nb]].reshape(len(pending), -1)
        sims = np.einsum("qrd,qd->qr", fb_pad[rows], q[pending],
                         optimize=True)
        sims[rows == N_TOTAL] = -np.inf
        still = []
        for j, b in enumerate(pending):
            o = np.lexsort((rows[j], -sims[j]))[:K]
            tK = sims[j][o[-1]]
            unsel = sel_sorted[b, nb] if nb < bm_flat.shape[1] else -np.inf
            if unsel + MARGIN < tK or nb >= bm_flat.shape[1]:
                topk_idx[b] = rows[j][o]
            else:
                still.append(b)
        pending = np.array(still, dtype=np.int64)
        nb *= 2
    return topk_idx


def _labels_to_output(topk_idx, target_bank):
    tb = np.asarray(target_bank).astype(np.int64)
    out = np.empty((B, NUM_CLASSES), dtype=np.int32)
    allc = np.arange(NUM_CLASSES)
    for b in range(B):
        mask = np.zeros(NUM_CLASSES, dtype=bool)
        mask[tb[topk_idx[b]]] = True
        out[b, :mask.sum()] = allc[mask]
        out[b, mask.sum():] = allc[~mask]
    return out


def kernel(query_feature, feature_bank, target_bank):
    query_feature = np.asarray(query_feature)
    feature_bank = np.asarray(feature_bank)
    target_bank = np.asarray(target_bank)
    bm, _ = _run_device(query_feature, feature_bank)
    topk_idx = _host_topk(bm, query_feature, feature_bank)
    return _labels_to_output(topk_idx, target_bank)

